# revision 24
# baseline (speedup 1.0000x reference)
"""Trainium2 Bass kernel for nn_DecoderBlockWithKeywords.

Decoder block: causal self-attn + gated (source-code / keywords) cross-attn
+ template cross-attn + FFN, with 4 LayerNorms.  B=4, T=1024, D=512, H=8,
dh=64, DFF=2048.

Sharding: pure data-parallel over (batch, query-half) -> 8 NeuronCores, no
collectives.  Each core holds all weights (fp16) and computes 512 query
tokens of one batch element.

Feature-major layout (X^T: [D on partitions, tokens on free]).  Score
matmuls S^T = K_h Q_h^T; softmax without max-subtraction; denominators via
a ones-column appended to V.  Attention-output normalization is DEFERRED:
the unnormalized AV^T is evicted to SBUF, per-head denominators are gathered
by one SBUF->SBUF DMA into [8,512], inverted with Ln/Exp, and broadcast to
all 128 partitions with a single K=8 selector matmul per feature chunk; the
normalize multiply happens right before the O-projection so the PE never
head-of-line blocks on the softmax tail.  A single manual activation-table
load (natural_log_exp_and_others) removes all run-time table switches.
Causal diagonal tiles restrict scores/exp/AV to the valid query range.  The
cc/ck gate is computed from the normalized attention outputs via
host-precomputed u = W_o @ gate_w vectors, the per-token gate scales are
folded into the attention outputs, and both O-projections accumulate into
one PSUM so the gated sum needs a single evict.  LayerNorm stats matmuls
are interleaved with the O-projection evicts; residuals are bf16.
Emission order interleaves independent K/V projections of later stages into
every softmax-normalize / LN latency window to keep the PE at its ramped
(2.4 GHz) p-state.

Programs are specialized at build time to the actual kv lengths; up to 8
distinct programs are compiled and launched concurrently on disjoint
device subsets.
"""

import os
import sys
import threading

import numpy as np

for _p in ("/opt/trn_rl_repo", "/root/.axon_site"):
    if os.path.isdir(_p) and _p not in sys.path:
        sys.path.append(_p)

import ml_dtypes
from contextlib import ExitStack

import concourse.bass as bass
import concourse.mybir as mybir
from concourse import bacc
from concourse.tile import TileContext

BF16 = np.float16
F32 = np.float32
NEG = -1000000.0
B, T, S, TM, KW, D, H, DFF = 4, 1024, 1024, 512, 64, 512, 8, 2048
DH = D // H  # 64
P = 128
NCH = D // P  # 4 feature chunks
AF = mybir.ActivationFunctionType
OP = mybir.AluOpType
ACT_TABLE_LN_EXP = 6  # natural_log_exp_and_others in act_info.json


# ---------------------------------------------------------------------------
# program builder
# ---------------------------------------------------------------------------

def build_program(qh, kts_cc, kts_ct, gate_b=0.0, apply_affine=False,
                  debug=False):
    """Build one core's Bass program.

    qh: 0/1 query half.  kts_cc/kts_ct: number of 128-wide kv tiles for the
    source-code / template cross attentions (specialized to actual length).
    """
    f32, bf16 = mybir.dt.float32, mybir.dt.float16
    KV = 512 * (qh + 1)          # self-attn kv range
    QOFF = qh * 512              # q columns inside xkvT

    nc = bacc.Bacc("TRN2", target_bir_lowering=False, debug=False)

    def din(name, shape, dt=bf16):
        return nc.dram_tensor(name, shape, dt, kind="ExternalInput").ap()

    xkvT = din("xkvT", [D, KV])
    srcT = din("srcT", [D, kts_cc * P])
    tmplT = din("tmplT", [D, kts_ct * P])
    kwT = din("kwT", [D, KW])
    wnames = [f"{n}_{p}" for n in ("sa", "cc", "ct", "ck")
              for p in ("wq", "wk", "wv", "wo")]
    wd = {n: din(n, [P, NCH * D]) for n in wnames}
    w1d = din("ffn_w1", [P, NCH * DFF])
    w2d = din("ffn_w2", [P, (DFF // P) * D])
    cb16d = din("cblob16", [P, P + 512 + 2 * NCH])
    cb32d = din("cblob32", [P, 3], f32)
    affine_d = din("ln_affine", [P, NCH * 8], f32) if apply_affine else None
    outT = nc.dram_tensor("outT", [D, 512], f32, kind="ExternalOutput").ap()
    dbg_outs = {}

    def mkdbg(nm, shape):
        if nm not in dbg_outs:
            dbg_outs[nm] = nc.dram_tensor(f"dbg_{nm}", shape, f32,
                                          kind="ExternalOutput").ap()
        return dbg_outs[nm]

    with TileContext(nc, pool_alloc_mode="queue") as tc, ExitStack() as ctx:
        # one act table covering Exp/Ln/Square/Relu/Copy: zero switches
        nc.scalar.add_instruction(mybir.InstLoadActFuncSet(
            name=nc.get_next_instruction_name(), ins=[], outs=[],
            act_func_set_id=ACT_TABLE_LN_EXP))

        pers = ctx.enter_context(tc.tile_pool(name="pers", bufs=1))
        # ---- persistent small constants (two packed blobs; DMAs are
        # issued AFTER the startup-critical weight/activation loads) -----
        cb16 = pers.tile([P, P + 512 + 2 * NCH], bf16, name="cb16_t")
        stair = cb16[:, 0:P]
        sel8 = cb16[0:H, P:P + 512]
        ucc_t = cb16[:, P + 512:P + 512 + NCH]
        uck_t = cb16[:, P + 512 + NCH:P + 512 + 2 * NCH]
        cb32 = pers.tile([P, 3], f32, name="cb32_t")
        ccbias = cb32[:, 0:1]
        ctbias = cb32[:, 1:2]
        kwbias = cb32[0:KW, 2:3]

        def load_consts():
            nc.sync.dma_start(out=cb16, in_=cb16d)
            nc.sync.dma_start(out=cb32, in_=cb32d)
        ones_b = pers.tile([P, 1], bf16, name="ones_b")
        nc.vector.memset(ones_b, 1.0)
        ones_row = pers.tile([1, P], bf16, name="ones_row")
        nc.vector.memset(ones_row, 1.0)
        eps_t = pers.tile([1, 1], f32, name="eps_t")
        nc.vector.memset(eps_t, 1e-5)
        gb_t = pers.tile([1, 1], f32, name="gb_t")
        nc.vector.memset(gb_t, -float(gate_b))
        affine = None
        if apply_affine:
            affine = pers.tile([P, NCH * 8], f32, name="affine_t")
            nc.sync.dma_start(out=affine, in_=affine_d)

        def tap(nm, tiles):
            if not debug:
                return
            cols = tiles[0].shape[-1]
            d = mkdbg(nm, [len(tiles) * P, cols])
            for i, t in enumerate(tiles):
                rows = t.shape[0]
                nc.gpsimd.dma_start(out=d[i * P:i * P + rows, :], in_=t)

        # ---- global shared pools --------------------------------------
        rpool = ctx.enter_context(tc.tile_pool(name="rpool", bufs=1))

        def mktiles(nm, cols=512, dt=bf16, n=NCH, tagp=None):
            tagp = tagp or nm
            return [rpool.tile([P, cols], dt, name=f"{nm}{i}", tag=f"{tagp}{i}",
                               bufs=1) for i in range(n)]

        smallp = ctx.enter_context(tc.tile_pool(name="smallp", bufs=1))
        trp = ctx.enter_context(tc.tile_pool(name="trp", bufs=1))
        psA = ctx.enter_context(tc.tile_pool(name="psA", bufs=2, space="PSUM"))
        psB = ctx.enter_context(tc.tile_pool(name="psB", bufs=2, space="PSUM"))

        _dma_rr = [0]

        def dma_load(out, in_):
            eng = (nc.sync, nc.scalar, nc.gpsimd)[_dma_rr[0] % 3]
            _dma_rr[0] += 1
            eng.dma_start(out=out, in_=in_)

        def load_w(pool, names):
            for n in names:
                wt[n] = pool.tile([P, NCH * D], bf16, name=f"{n}_t",
                                  tag=f"{n}_t", bufs=1)
                dma_load(wt[n], wd[n])
        wt = {}

        def w_lhsT(n, i, j):
            return wt[n][:, i * D + j * P: i * D + (j + 1) * P]

        def w_rhs(n, i, cols=D):
            return wt[n][:, i * D: i * D + cols]

        def load_act(pool, nm, dram_ap, cols):
            tiles = []
            for i in range(NCH):
                t = pool.tile([P, cols], bf16, name=f"{nm}{i}",
                              tag=f"{nm}{i}", bufs=1)
                dma_load(t, dram_ap[i * P:(i + 1) * P, :])
                tiles.append(t)
            return tiles

        # alternating evict engines to balance DVE vs ACT; 'act' mode
        # keeps filler evicts off the DVE queue inside LN windows
        ev_ctr = [0]
        ev_mode = ["alt"]

        def ev_copy(out_ap, in_ap):
            if ev_mode[0] == "act" or ev_ctr[0] % 2:
                nc.scalar.activation(out_ap, in_ap, AF.Copy)
            else:
                nc.vector.tensor_copy(out_ap, in_ap)
            ev_ctr[0] += 1

        # ----------------------------------------------------------------
        # helpers
        # ----------------------------------------------------------------
        def proj_fm_thunks(wn, rhs_tiles, ncols, out_tiles, evict):
            ntt = (ncols + 511) // 512
            thunks = []
            for j in range(NCH):
                for t in range(ntt):
                    def go(j=j, t=t):
                        cs = t * 512
                        ce = min(ncols, cs + 512)
                        ps = psA.tile([P, ce - cs], mybir.dt.float32,
                                      name="proj_ps", tag="pps")
                        for i in range(NCH):
                            nc.tensor.matmul(ps, w_lhsT(wn, i, j),
                                             rhs_tiles[i][:, cs:ce],
                                             start=(i == 0),
                                             stop=(i == NCH - 1))
                        evict(j, cs, ce, ps, out_tiles)
                    thunks.append(go)
            return thunks

        def proj_fm(wn, rhs_tiles, ncols, out_tiles, evict):
            for th in proj_fm_thunks(wn, rhs_tiles, ncols, out_tiles, evict):
                th()

        def evict_copy(j, cs, ce, ps, out_tiles):
            ev_copy(out_tiles[j][:, cs:ce], ps)

        def proj_v_thunks(enc_tiles, wn, nkv, vt_list, vpool, ktag):
            nch_tok = (nkv + P - 1) // P
            vt_list.extend(
                vpool.tile([min(P, nkv - m * P), H * (DH + 1)], bf16,
                           name=f"{ktag}_v{m}", tag=f"{ktag}_v{m}", bufs=1)
                for m in range(nch_tok))
            thunks = []
            for m in range(nch_tok):
                def go(m=m):
                    rows = min(P, nkv - m * P)
                    ps = psA.tile([rows, D], mybir.dt.float32,
                                  name="v_ps", tag="pps")
                    for i in range(NCH):
                        nc.tensor.matmul(ps,
                                         enc_tiles[i][:, m * P:m * P + rows],
                                         w_rhs(wn, i),
                                         start=(i == 0), stop=(i == NCH - 1))
                    vt = vt_list[m]
                    src3 = ps.rearrange("p (g c) -> p g c", c=DH)
                    dst3 = vt.rearrange("p (g c) -> p g c", c=DH + 1)
                    ev_copy(dst3[:, :, 0:DH], src3)
                    nc.gpsimd.memset(dst3[:, :, DH:DH + 1], 1.0)
                thunks.append(go)
            return thunks

        def proj_v(enc_tiles, wn, nkv, vt_list, vpool, ktag):
            for th in proj_v_thunks(enc_tiles, wn, nkv, vt_list, vpool, ktag):
                th()

        def attention_core(qt, kt, vt_list, at_u, dn8, bias_tile, causal,
                           ktag, fill=None):
            """Scores + AV for all head pairs, AV software-pipelined two kv
            tiles behind the scores so it never waits on the exp.  Leaves
            UNNORMALIZED attention^T in at_u (bf16) and raw per-head softmax
            denominators in dn8 [1, H*512].  `fill` is a list of PE-work
            thunks drained at head-pair boundaries."""
            nkt = len(vt_list)
            fill = fill or []
            for hp in range(H // 2):
                po = [psB.tile([DH + 1, 512], mybir.dt.float32,
                               name=f"{ktag}_po{s}", tag="po")
                      for s in range(2)]
                pend = []

                def flush_one(hp=hp, po=po, pend=pend):
                    kt_i, pt, q0 = pend.pop(0)
                    for s in range(2):
                        h = 2 * hp + s
                        nc.tensor.matmul(
                            po[s][:, q0:],
                            vt_list[kt_i][:, h * (DH + 1):
                                          h * (DH + 1) + DH + 1],
                            pt[:, s * 512 + q0:(s + 1) * 512],
                            start=(kt_i == 0), stop=(kt_i == nkt - 1))
                for kt_i in range(nkt):
                    rows = vt_list[kt_i].shape[0]
                    d = kt_i - (nkt - 4) if causal else -1
                    q0 = d * P if (causal and d > 0) else 0
                    ps = psB.tile([rows, 1024], mybir.dt.float32,
                                  name=f"{ktag}_sc", tag="sc")
                    pt = trp.tile([rows, 1024], bf16,
                                  name=f"{ktag}_pt", tag="pt", bufs=4)
                    for s in range(2):
                        ro = s * DH
                        nc.tensor.matmul(
                            ps[:, s * 512 + q0:(s + 1) * 512],
                            kt[hp][ro:ro + DH, kt_i * P:kt_i * P + rows],
                            qt[hp][ro:ro + DH, q0:], start=True, stop=True)
                    if causal and d >= 0:
                        for s in range(2):
                            o = s * 512
                            nc.vector.tensor_add(
                                ps[:, o + d * P:o + (d + 1) * P],
                                ps[:, o + d * P:o + (d + 1) * P], stair)
                            nc.scalar.activation(
                                pt[:, o + q0:o + 512], ps[:, o + q0:o + 512],
                                AF.Exp, scale=0.125)
                    else:
                        bias = 0.0
                        if bias_tile is not None and kt_i == nkt - 1:
                            bias = bias_tile[:rows, :]
                        nc.scalar.activation(pt, ps, AF.Exp,
                                             bias=bias, scale=0.125)
                    pend.append((kt_i, pt, q0))
                    if len(pend) > 2:
                        flush_one()
                while pend:
                    flush_one()
                # evacuate AV + denominators
                for s in range(2):
                    h = 2 * hp + s
                    ro = s * DH
                    nc.vector.tensor_copy(at_u[hp][ro:ro + DH, :],
                                          po[s][0:DH, :])
                    nc.vector.tensor_copy(dn8[:, h * 512:(h + 1) * 512],
                                          po[s][DH:DH + 1, :])
                # drain filler PE work into the softmax latency window
                n_emit = -(-len(fill) // (H // 2 - hp))
                for _ in range(n_emit):
                    fill.pop(0)()

        def normalize_tail(at_u, dn8, ktag):
            """1/n via Ln/Exp on the gathered [8,512] denominators, K=8
            selector broadcast, in-place scale of at_u."""
            nmat = smallp.tile([H, 512], bf16,
                               name=f"{ktag}_nmat", tag="nmat", bufs=1)
            nc.gpsimd.dma_start(
                out=nmat, in_=dn8.rearrange("p (h c) -> p h c", c=512))
            lnn = smallp.tile([H, 512], mybir.dt.float32,
                              name=f"{ktag}_lnn", tag="lnn", bufs=1)
            nc.scalar.activation(lnn, nmat, AF.Ln)
            ninv8 = smallp.tile([H, 512], bf16,
                                name=f"{ktag}_ninv8", tag="ninv8", bufs=1)
            nc.scalar.activation(ninv8, lnn, AF.Exp, scale=-1.0)
            for c in range(NCH):
                bc = psB.tile([P, 512], mybir.dt.float32,
                              name=f"{ktag}_bc", tag="sc")
                nc.tensor.matmul(bc, sel8[:, c * P:(c + 1) * P], ninv8,
                                 start=True, stop=True)
                nc.vector.tensor_mul(at_u[c], at_u[c], bc)

        def mk_dn8(ktag):
            return trp.tile([1, H * 512], bf16,
                            name=f"{ktag}_dn8", tag="dn8", bufs=1)

        # --- LayerNorm split into stats (interleaved with evicts) + finish
        def ln_stats_psums():
            ps_s = psB.tile([1, 512], mybir.dt.float32,
                            name="ln_ps_s", tag="sc")
            ps_q = psB.tile([1, 512], mybir.dt.float32,
                            name="ln_ps_q", tag="sc")
            return ps_s, ps_q

        def ln_stat_chunk(ps_s, ps_q, r_tile, j):
            nc.tensor.matmul(ps_s, ones_b, r_tile,
                             start=(j == 0), stop=(j == NCH - 1))
            sq = trp.tile([P, 512], bf16, name="ln_sq", tag="ln_sq", bufs=4)
            nc.scalar.activation(sq, r_tile, AF.Square)
            nc.tensor.matmul(ps_q, ones_b, sq,
                             start=(j == 0), stop=(j == NCH - 1))

        def ln_finish(ps_s, ps_q, r_tiles, out_tiles, ln_idx, filler=None):
            mean16 = smallp.tile([1, 512], bf16,
                                 name="ln_mean16", tag="ln_stat", bufs=4)
            nc.vector.tensor_scalar_mul(mean16, ps_s, 1.0 / D)
            msq = smallp.tile([1, 512], mybir.dt.float32,
                              name="ln_msq", tag="ln_stat", bufs=4)
            nc.scalar.activation(msq, ps_s, AF.Square, scale=1.0 / D)
            var = smallp.tile([1, 512], mybir.dt.float32,
                              name="ln_var", tag="ln_stat", bufs=4)
            nc.vector.scalar_tensor_tensor(var, ps_q, 1.0 / D, msq,
                                           op0=OP.mult, op1=OP.subtract)
            lnv = smallp.tile([1, 512], mybir.dt.float32,
                              name="ln_lnv", tag="ln_stat", bufs=4)
            nc.scalar.activation(lnv, var, AF.Ln, bias=eps_t[:, :])
            rstd = smallp.tile([1, 512], bf16,
                               name="ln_rstd", tag="ln_stat", bufs=4)
            nc.scalar.activation(rstd, lnv, AF.Exp, scale=-0.5)
            meanb = psB.tile([P, 512], mybir.dt.float32,
                             name="ln_meanb", tag="sc")
            nc.tensor.matmul(meanb, ones_row, mean16, start=True, stop=True)
            rstdb = psB.tile([P, 512], mybir.dt.float32,
                             name="ln_rstdb", tag="sc")
            nc.tensor.matmul(rstdb, ones_row, rstd, start=True, stop=True)
            if filler:
                ev_mode[0] = "act"
                for th in filler:
                    th()
                ev_mode[0] = "alt"
            for j in range(NCH):
                tmp = trp.tile([P, 512], bf16,
                               name="ln_tmp", tag="ln_tmp", bufs=2)
                nc.vector.tensor_sub(tmp, r_tiles[j], meanb)
                nc.vector.tensor_mul(out_tiles[j], tmp, rstdb)
                if apply_affine:
                    g = affine[:, ln_idx * 2 * NCH + j:
                               ln_idx * 2 * NCH + j + 1]
                    b = affine[:, ln_idx * 2 * NCH + NCH + j:
                               ln_idx * 2 * NCH + NCH + j + 1]
                    nc.vector.tensor_scalar(out_tiles[j], out_tiles[j],
                                            g, b, op0=OP.mult, op1=OP.add)

        # ================================================================
        # emission (ordered for cross-stage overlap)
        # ================================================================
        r1 = mktiles("r1", tagp="rA")
        y = mktiles("y", tagp="lnA")
        r2 = mktiles("r2", tagp="rB")
        z = mktiles("z", tagp="lnB")

        # ct pool opened first (outlives the sa/cc phase pools; LIFO close)
        ctwsb = ctx.enter_context(tc.tile_pool(name="ctw_sb", bufs=1))
        ccsb_cm = tc.tile_pool(name="cc_sb", bufs=1)
        ccsb = ccsb_cm.__enter__()
        sasb_cm = tc.tile_pool(name="sa_sb", bufs=1)
        sasb = sasb_cm.__enter__()

        # --- stage 1: self attention ---
        load_w(sasb, ["sa_wk"])
        xkv = load_act(sasb, "xkv", xkvT, KV)
        load_w(sasb, ["sa_wv", "sa_wq", "sa_wo"])
        # cc/ck weights + activations: queue DMAs early
        load_w(ccsb, ["cc_wk", "cc_wv", "ck_wk", "ck_wv",
                      "cc_wq", "ck_wq", "cc_wo", "ck_wo"])
        srcl = load_act(ccsb, "src", srcT, kts_cc * P)
        kwe = load_act(ccsb, "kw", kwT, KW)
        load_consts()

        qt = [sasb.tile([P, 512], bf16, name=f"sa_q{i}", tag=f"sa_q{i}",
                        bufs=1) for i in range(NCH)]
        ktl = [sasb.tile([P, KV], bf16, name=f"sa_k{i}", tag=f"sa_k{i}",
                         bufs=1) for i in range(NCH)]
        xq = [t[:, QOFF:QOFF + 512] for t in xkv]
        proj_fm("sa_wk", xkv, KV, ktl, evict_copy)
        vts = []
        proj_v(xkv, "sa_wv", KV, vts, sasb, "sa")
        proj_fm("sa_wq", xq, 512, qt, evict_copy)
        at_sa = [trp.tile([P, 512], bf16, name=f"sa_at{i}", tag=f"atA{i}",
                          bufs=1) for i in range(NCH)]
        sa_dn8 = mk_dn8("sa")
        cc_kt = [ccsb.tile([P, kts_cc * P], bf16, name=f"cc_k{i}",
                           tag=f"cc_k{i}", bufs=1) for i in range(NCH)]
        sa_fill = proj_fm_thunks("cc_wk", srcl, kts_cc * P, cc_kt, evict_copy)
        attention_core(qt, ktl, vts, at_sa, sa_dn8, None, True, "sa",
                       fill=sa_fill)

        # ct weights + template: queue DMAs now (pool lives to the end)
        load_w(ctwsb, ["ct_wk", "ct_wv", "ct_wq", "ct_wo"])
        tmpl = load_act(ctwsb, "tmpl", tmplT, kts_ct * P)

        normalize_tail(at_sa, sa_dn8, "sa")
        tap("sa_at", at_sa)

        ln1_s, ln1_q = ln_stats_psums()

        def evict_resid_x(j, cs, ce, ps, out_tiles):
            nc.vector.tensor_add(out_tiles[j][:, cs:ce], ps, xq[j])
            ln_stat_chunk(ln1_s, ln1_q, out_tiles[j], j)
        proj_fm("sa_wo", at_sa, 512, r1, evict_resid_x)
        tap("r1", r1)

        # filler for LN1 latency: ck K proj before the chain, V projs after
        ck_kt = [ccsb.tile([P, KW], bf16, name=f"ck_k{i}", tag=f"ck_k{i}",
                           bufs=1) for i in range(NCH)]
        proj_fm("ck_wk", kwe, KW, ck_kt, evict_copy)
        ck_vts = []
        cc_vts = []
        ln1_fill = (proj_v_thunks(kwe, "ck_wv", KW, ck_vts, ccsb, "ck")
                    + proj_v_thunks(srcl, "cc_wv", kts_cc * P, cc_vts, ccsb,
                                    "cc"))
        ln_finish(ln1_s, ln1_q, r1, y, 0, filler=ln1_fill)
        tap("y", y)

        # --- stage 2: cc + ck cross attention + gate ---
        cc_qt = [ccsb.tile([P, 512], bf16, name=f"cc_q{i}", tag=f"cc_q{i}",
                           bufs=1) for i in range(NCH)]
        proj_fm("cc_wq", y, 512, cc_qt, evict_copy)
        at_cc = [ccsb.tile([P, 512], bf16, name=f"cc_at{i}", tag=f"atC{i}",
                           bufs=1) for i in range(NCH)]
        cc_dn8 = mk_dn8("cc")
        ct_kt = [ctwsb.tile([P, kts_ct * P], bf16, name=f"ct_k{i}",
                            tag=f"ct_k{i}", bufs=1) for i in range(NCH)]
        cc_fill = proj_fm_thunks("ct_wk", tmpl, kts_ct * P, ct_kt, evict_copy)
        attention_core(cc_qt, cc_kt, cc_vts, at_cc, cc_dn8, ccbias, False,
                       "cc", fill=cc_fill)
        # filler for cc normalize: ck Q projection
        ck_qt = [ccsb.tile([P, 512], bf16, name=f"ck_q{i}", tag=f"ck_q{i}",
                           bufs=1) for i in range(NCH)]
        proj_fm("ck_wq", y, 512, ck_qt, evict_copy)
        normalize_tail(at_cc, cc_dn8, "cc")
        at_ck = [ccsb.tile([P, 512], bf16, name=f"ck_at{i}", tag=f"atK{i}",
                           bufs=1) for i in range(NCH)]
        ck_dn8 = mk_dn8("ck")
        attention_core(ck_qt, ck_kt, ck_vts, at_ck, ck_dn8, kwbias, False,
                       "ck")
        sasb_cm.__exit__(None, None, None)
        normalize_tail(at_ck, ck_dn8, "ck")
        tap("cc_at", at_cc)
        tap("ck_at", at_ck)

        # --- gate (from normalized at via u = W_o @ gate_w) ---
        ps_g = psB.tile([1, 512], mybir.dt.float32, name="gate_ps", tag="sc")
        for i in range(NCH):
            nc.tensor.matmul(ps_g, ucc_t[:, i:i + 1], at_cc[i],
                             start=(i == 0), stop=False)
        for i in range(NCH):
            nc.tensor.matmul(ps_g, uck_t[:, i:i + 1], at_ck[i],
                             start=False, stop=(i == NCH - 1))
        ge = smallp.tile([1, 512], mybir.dt.float32, name="gate_e",
                         tag="gate_edg", bufs=3)
        nc.scalar.activation(ge, ps_g, AF.Exp, scale=-1.0, bias=gb_t[:, :])
        gl2 = smallp.tile([1, 512], mybir.dt.float32, name="gate_lnd",
                          tag="gate_edg", bufs=3)
        nc.scalar.activation(gl2, ge, AF.Ln, bias=1.0)
        gg = smallp.tile([1, 512], bf16, name="gate_g",
                         tag="gate_edg", bufs=3)
        nc.scalar.activation(gg, gl2, AF.Exp, scale=-1.0)
        gm = smallp.tile([1, 512], bf16, name="gate_m",
                         tag="gate_edg", bufs=3)
        nc.vector.tensor_scalar(gm, gg, -1.0, 1.0, op0=OP.mult, op1=OP.add)
        # filler during gate chain: ct V projection
        ct_vts = []
        proj_v(tmpl, "ct_wv", kts_ct * P, ct_vts, ctwsb, "ct")
        ggb = psB.tile([P, 512], mybir.dt.float32, name="gate_gb", tag="sc")
        nc.tensor.matmul(ggb, ones_row, gg, start=True, stop=True)
        gmb = psB.tile([P, 512], mybir.dt.float32, name="gate_mb", tag="sc")
        nc.tensor.matmul(gmb, ones_row, gm, start=True, stop=True)
        for j in range(NCH):
            nc.vector.tensor_mul(at_cc[j], at_cc[j], ggb)
            nc.vector.tensor_mul(at_ck[j], at_ck[j], gmb)

        # fused O-projection: r2 = y + Wo_cc^T at_cc_g + Wo_ck^T at_ck_g
        ln2_s, ln2_q = ln_stats_psums()
        for j in range(NCH):
            ps = psA.tile([P, 512], mybir.dt.float32, name="o2_ps",
                          tag="pps")
            for i in range(NCH):
                nc.tensor.matmul(ps, w_lhsT("cc_wo", i, j), at_cc[i],
                                 start=(i == 0), stop=False)
            for i in range(NCH):
                nc.tensor.matmul(ps, w_lhsT("ck_wo", i, j), at_ck[i],
                                 start=False, stop=(i == NCH - 1))
            nc.vector.tensor_add(r2[j], ps, y[j])
            ln_stat_chunk(ln2_s, ln2_q, r2[j], j)
        tap("r2", r2)
        ccsb_cm.__exit__(None, None, None)
        # ffn weights: queue DMAs now (space freed by cc pool)
        ffsb = ctx.enter_context(tc.tile_pool(name="ff_sb", bufs=1))
        w1t = ffsb.tile([P, NCH * DFF], bf16, name="w1_t", tag="w1_t")
        dma_load(w1t, w1d)
        w2t = ffsb.tile([P, (DFF // P) * D], bf16, name="w2_t", tag="w2_t")
        dma_load(w2t, w2d)
        ln_finish(ln2_s, ln2_q, r2, z, 1)
        tap("z", z)

        # --- stage 3: ct cross attention ---
        r3 = mktiles("r3", tagp="rA")
        ze = mktiles("ze", tagp="lnA")
        ct_qt = [ctwsb.tile([P, 512], bf16, name=f"ct_q{i}", tag=f"ct_q{i}",
                            bufs=1) for i in range(NCH)]
        proj_fm("ct_wq", z, 512, ct_qt, evict_copy)
        at_ct = [trp.tile([P, 512], bf16, name=f"ct_at{i}", tag=f"atA{i}",
                          bufs=1) for i in range(NCH)]
        ct_dn8 = mk_dn8("ct")
        attention_core(ct_qt, ct_kt, ct_vts, at_ct, ct_dn8, ctbias, False,
                       "ct")
        normalize_tail(at_ct, ct_dn8, "ct")
        tap("ct_at", at_ct)

        ln3_s, ln3_q = ln_stats_psums()

        def evict_resid_z(j, cs, ce, ps, out_tiles):
            nc.vector.tensor_add(out_tiles[j][:, cs:ce], ps, z[j])
            ln_stat_chunk(ln3_s, ln3_q, out_tiles[j], j)
        proj_fm("ct_wo", at_ct, 512, r3, evict_resid_z)
        tap("r3", r3)
        ln_finish(ln3_s, ln3_q, r3, ze, 2)
        tap("ze", ze)

        # --- stage 4: FFN ---
        ht = [ffsb.tile([P, 512], bf16, name=f"ff_h{i}", tag=f"ff_h{i}",
                        bufs=1) for i in range(DFF // P)]
        for jf in range(DFF // P):
            ps = psA.tile([P, 512], mybir.dt.float32, name="ff_ps",
                          tag="pps")
            for i in range(NCH):
                nc.tensor.matmul(ps, w1t[:, i * DFF + jf * P:
                                         i * DFF + (jf + 1) * P],
                                 ze[i], start=(i == 0), stop=(i == NCH - 1))
            if jf % 2 == 0:
                nc.scalar.activation(ht[jf], ps, AF.Relu)
            else:
                nc.vector.tensor_scalar_max(ht[jf], ps, 0.0)
        r4 = mktiles("r4", tagp="rB")
        ln4_s, ln4_q = ln_stats_psums()
        for j in range(NCH):
            ps = psA.tile([P, 512], mybir.dt.float32, name="ff_ps2",
                          tag="pps")
            for i in range(DFF // P):
                nc.tensor.matmul(ps, w2t[:, i * D + j * P: i * D + (j + 1) * P],
                                 ht[i], start=(i == 0),
                                 stop=(i == DFF // P - 1))
            nc.vector.tensor_add(r4[j], ps, ze[j])
            ln_stat_chunk(ln4_s, ln4_q, r4[j], j)
        fin = [ffsb.tile([P, 512], mybir.dt.float32, name=f"fin{i}",
                         tag="fin", bufs=2) for i in range(NCH)]
        ln_finish(ln4_s, ln4_q, r4, fin, 3)
        for j in range(NCH):
            nc.sync.dma_start(out=outT[j * P:(j + 1) * P, :], in_=fin[j])

    nc.compile()
    return nc


# ---------------------------------------------------------------------------
# host-side input preparation
# ---------------------------------------------------------------------------

def _prep_shared(inputs):
    """Cast/transform weights shared by every core."""
    sh = {}

    def wlay(w, nch):  # [(i p), n] -> [p, i*n] contiguous per partition
        n = w.shape[1]
        return np.ascontiguousarray(
            w.reshape(nch, P, n).transpose(1, 0, 2).reshape(P, nch * n)
            .astype(BF16))
    for n in ("sa", "cc", "ct", "ck"):
        for p in ("wq", "wk", "wv", "wo"):
            sh[f"{n}_{p}"] = wlay(inputs[f"{n}_{p}"].astype(F32), NCH)
    sh["ffn_w1"] = wlay(inputs["ffn_w1"].astype(F32), NCH)
    sh["ffn_w2"] = wlay(inputs["ffn_w2"].astype(F32), DFF // P)
    gw = inputs["gate_w"].astype(F32)
    u_cc = inputs["cc_wo"].astype(F32) @ gw[:D, 0]
    u_ck = inputs["ck_wo"].astype(F32) @ gw[D:, 0]
    sh["u_cc"] = u_cc.reshape(NCH, P).T.astype(BF16)
    sh["u_ck"] = u_ck.reshape(NCH, P).T.astype(BF16)
    kl, ql = np.arange(P)[:, None], np.arange(P)[None, :]
    stair = np.where(kl <= ql, 0.0, NEG).astype(BF16)
    sel = np.zeros((P, 512), BF16)
    for c in range(NCH):
        sel[2 * c, c * P:c * P + DH] = 1.0
        sel[2 * c + 1, c * P + DH:(c + 1) * P] = 1.0
    ucc_l = np.zeros((P, NCH), BF16)
    ucc_l[:, :] = sh.pop("u_cc")
    uck_l = np.zeros((P, NCH), BF16)
    uck_l[:, :] = sh.pop("u_ck")
    sh["cblob16"] = np.ascontiguousarray(
        np.concatenate([stair, sel, ucc_l, uck_l], axis=1))
    return sh


def _len_bias(L, kts, width=P):
    """[width,1] f32 additive bias for the LAST kv tile."""
    base = (kts - 1) * P
    idx = base + np.arange(width)
    return np.where(idx < L, 0.0, NEG).astype(F32)[:, None]


def _prep_core(inputs, sh, b, qh, kts_cc, kts_ct):
    KVn = 512 * (qh + 1)
    m = dict(sh)
    xT = np.ascontiguousarray(inputs["x"][b].T.astype(BF16))  # [D, T]
    m["xkvT"] = np.ascontiguousarray(xT[:, :KVn])
    Ls = int(inputs["source_code_len"][b])
    st = np.zeros((D, kts_cc * P), BF16)
    st[:, :Ls] = inputs["source_code_enc"][b, :Ls].T.astype(BF16)
    m["srcT"] = st
    Lt = int(inputs["template_len"][b])
    tt = np.zeros((D, kts_ct * P), BF16)
    tt[:, :Lt] = inputs["template_enc"][b, :Lt].T.astype(BF16)
    m["tmplT"] = tt
    m["kwT"] = np.ascontiguousarray(inputs["keywords_enc"][b].T.astype(BF16))
    cb32 = np.zeros((P, 3), F32)
    cb32[:, 0:1] = _len_bias(Ls, kts_cc)
    cb32[:, 1:2] = _len_bias(Lt, kts_ct)
    cb32[:KW, 2:3] = _len_bias(int(inputs["keywords_len"][b]), 1, KW)
    m["cblob32"] = cb32
    return m


# ---------------------------------------------------------------------------
# concurrent multi-program PJRT runner (adapted from bass2jax.run_bass_via_pjrt)
# ---------------------------------------------------------------------------

def _run_groups(groups):
    """groups: list of (nc, core_ids, in_maps).  Dispatch all groups onto
    their own device subsets, then gather.  Returns {core_id: {name: arr}}."""
    import jax
    import numpy as _np
    from jax.sharding import Mesh, PartitionSpec
    from jax.experimental.shard_map import shard_map
    from concourse import bass2jax
    from concourse.bass2jax import (_bass_exec_p, install_neuronx_cc_hook,
                                    partition_id_tensor)

    install_neuronx_cc_hook()
    devices = jax.devices()

    def make_launch(nc, core_ids, in_maps):
        pname = (nc.partition_id_tensor.name
                 if nc.partition_id_tensor else None)
        in_names, out_names, out_avals, zero_outs = [], [], [], []
        for alloc in nc.m.functions[0].allocations:
            if not isinstance(alloc, mybir.MemoryLocationSet):
                continue
            name = alloc.memorylocations[0].name
            if alloc.kind == "ExternalInput":
                if name == pname:
                    continue
                in_names.append(name)
            elif alloc.kind == "ExternalOutput":
                shape = tuple(alloc.tensor_shape)
                dtype = mybir.dt.np(alloc.dtype)
                out_names.append(name)
                out_avals.append(jax.core.ShapedArray(shape, dtype))
                zero_outs.append(_np.zeros(shape, dtype))
        n_params, n_outs = len(in_names), len(out_avals)
        all_in_names = in_names + out_names
        if pname is not None:
            all_in_names = all_in_names + [pname]

        def _body(*args):
            operands = list(args)
            if pname is not None:
                operands.append(partition_id_tensor())
            outs = _bass_exec_p.bind(
                *operands, out_avals=tuple(out_avals),
                in_names=tuple(all_in_names), out_names=tuple(out_names),
                lowering_input_output_aliases=(),
                sim_require_finite=False, sim_require_nnan=False, nc=nc)
            return tuple(outs)

        donate = tuple(range(n_params, n_params + n_outs))
        devs = [devices[c] for c in core_ids]
        if len(core_ids) == 1:
            fn = jax.jit(_body, donate_argnums=donate, keep_unused=True,
                         device=devs[0])
            args = [in_maps[0][nm] for nm in in_names] + list(zero_outs)
            out_arrs = fn(*args)
            return out_names, out_avals, out_arrs, None
        mesh = Mesh(_np.asarray(devs), ("core",))
        in_specs = (PartitionSpec("core"),) * (n_params + n_outs)
        out_specs = (PartitionSpec("core"),) * n_outs
        fn = jax.jit(shard_map(_body, mesh=mesh, in_specs=in_specs,
                               out_specs=out_specs, check_rep=False),
                     donate_argnums=donate, keep_unused=True)
        cat = [_np.concatenate([_np.asarray(m[nm]) for m in in_maps], axis=0)
               for nm in in_names]
        catz = [_np.zeros((len(core_ids) * z.shape[0], *z.shape[1:]), z.dtype)
                for z in zero_outs]
        out_arrs = fn(*cat, *catz)
        return out_names, out_avals, out_arrs, len(core_ids)

    last_err = None
    for _attempt in range(3):
        try:
            launched = []
            for nc, core_ids, in_maps in groups:
                launched.append((core_ids, make_launch(nc, core_ids, in_maps)))
            results = {}
            for core_ids, (out_names, out_avals, out_arrs, ncores) in launched:
                if ncores is None:
                    results[core_ids[0]] = {nm: _np.asarray(out_arrs[i])
                                            for i, nm in enumerate(out_names)}
                else:
                    for ci, c in enumerate(core_ids):
                        results[c] = {
                            nm: _np.asarray(out_arrs[i]).reshape(
                                ncores, *out_avals[i].shape)[ci]
                            for i, nm in enumerate(out_names)}
            return results
        except Exception as e:  # transient NRT device errors: retry
            last_err = e
            import time as _time
            _time.sleep(2.0)
    raise last_err


_PROGRAM_CACHE = {}
_CACHE_LOCK = threading.Lock()


def _get_program(key):
    with _CACHE_LOCK:
        if key in _PROGRAM_CACHE:
            return _PROGRAM_CACHE[key]
    qh, kts_cc, kts_ct, gate_b, aff = key
    nc = build_program(qh, kts_cc, kts_ct, gate_b=gate_b, apply_affine=aff)
    with _CACHE_LOCK:
        _PROGRAM_CACHE[key] = nc
    return nc


# ---------------------------------------------------------------------------
# entry point
# ---------------------------------------------------------------------------

def kernel(**inputs):
    inputs = {k: np.asarray(v) for k, v in inputs.items()}
    gate_b = float(inputs["gate_b"].reshape(-1)[0])
    aff = not all(
        np.all(inputs[f"ln{j}_g"] == 1.0) and np.all(inputs[f"ln{j}_b"] == 0.0)
        for j in range(1, 5))
    affine_arr = None
    if aff:
        affine_arr = np.zeros((P, NCH * 8), F32)
        for ln in range(4):
            g = inputs[f"ln{ln + 1}_g"].astype(F32).reshape(NCH, P).T
            bb = inputs[f"ln{ln + 1}_b"].astype(F32).reshape(NCH, P).T
            affine_arr[:, ln * 2 * NCH: ln * 2 * NCH + NCH] = g
            affine_arr[:, ln * 2 * NCH + NCH: (ln + 1) * 2 * NCH] = bb

    sh = _prep_shared(inputs)
    # core -> (program key, in_map)
    core_keys, core_maps = [], []
    for c in range(8):
        b, qh = c // 2, c % 2
        kts_cc = max(1, -(-int(inputs["source_code_len"][b]) // P))
        kts_ct = max(1, -(-int(inputs["template_len"][b]) // P))
        key = (qh, kts_cc, kts_ct, gate_b, aff)
        m = _prep_core(inputs, sh, b, qh, kts_cc, kts_ct)
        if aff:
            m["ln_affine"] = affine_arr
        core_keys.append(key)
        core_maps.append(m)

    # build distinct programs (parallel threads: walrus compile is subprocess)
    distinct = sorted(set(core_keys))
    threads = [threading.Thread(target=_get_program, args=(k,))
               for k in distinct]
    for t in threads:
        t.start()
    for t in threads:
        t.join()

    groups = []
    for key in distinct:
        cores = [c for c in range(8) if core_keys[c] == key]
        groups.append((_get_program(key), cores, [core_maps[c] for c in cores]))

    results = _run_groups(groups)

    out = np.empty((B, T, D), np.float32)
    for c in range(8):
        b, qh = c // 2, c % 2
        out[b, qh * 512:(qh + 1) * 512, :] = results[c]["outT"].T
    return out
